# revision 16
# baseline (speedup 1.0000x reference)
"""CLIP loss (nn_ClipLossAcc) on 8 Trainium2 NeuronCores.

Strategy (data-parallel over rows, chunked CLIP loss):
  - Shard the N=16384 rows across 8 cores (2048 rows each); each core computes
    its 2048 x 16384 logits slice in 2048-column groups held in PSUM.
  - Matmuls in fp8 e4m3 with perf_mode=DoubleRow (2 K-tiles per pass); e4m3
    quantization of randn features costs ~8e-4 relative loss error.
  - exp(l - C) with fixed offset C=100 (max logit ~= 100); the elementwise
    wall is split between ACT (exact exp via LUT, row sums fused via
    accum_out) and DVE (Schraudolph bf16 exp for SCH_RT row-tiles:
    u16(x*(2^7/ln2) + (127-c)*128) bitcast to bf16; the f32->u16 convert
    saturates negatives to 0 -- verified on HW -- so no clamp is needed;
    c=0.0575 zero-centers the +-4% multiplicative error).
  - Column accumulation: exp tiles are summed into S=4 strided partial
    accumulators per group (round-robin rt%4) so consecutive DVE adds are
    independent -- in-place chains on one accumulator measure ~5us/op on HW
    while independent adds run at ~0.9us (2x mode).  DVE work is further
    software-pipelined: each tile's reduce lags one tile, its add two tiles,
    so back-to-back dependent DVE instructions never stall the in-order
    queue.  Optionally some tiles' adds run on gpsimd (pool_rt) or as
    gpsimd-initiated accumulating DMAs (dma_rt) into separate partials.
  - Column sums over the 128 partitions via a ones-vector bf16 matmul,
    staged through SBUF on ACT (idle during that tail) and DMA'd out.
  - Diagonal logits extracted exactly from PSUM with an identity-mask
    multiply+reduce; per-core text features pre-rotated so the diagonal
    block lands in local column-group 0 (SPMD-identical program).

Final host combine:  loss = C + (0.5*(sum_i log rowsum_i + sum_j log colsum_j)
                                 - sum_i diag_i) / N
"""

import math

import numpy as np
import ml_dtypes

import concourse.bass as bass
import concourse.tile as tile
from concourse import bacc, mybir
from concourse.bass_utils import run_bass_kernel_spmd

N_CORES = 8
C_OFF = 100.0

# Schraudolph bf16 exp constants
A16 = 128.0 / math.log(2.0)
B16 = (127.0 - 0.05753) * 128.0

# row-tiles whose exp runs on DVE (Schraudolph) instead of ACT
SCH_RT = frozenset((3, 7, 11))
# row-tiles whose column-accumulate add runs on gpsimd (Pool): measured
# ~11.6us/add on HW -- unusable, keep empty
POOL_RT = frozenset()
# row-tiles whose column-accumulate runs as accumulating DMA (~2.6us/add on
# HW, but on otherwise-idle DMA engines)
DMA_RT = frozenset((1, 2, 5, 6, 9, 10, 13, 14))

_NC_CACHE = {}


def build_nc(N, D, repeat=1, sch_rt=SCH_RT, pool_rt=POOL_RT, dma_rt=DMA_RT):
    key = (N, D, repeat, tuple(sorted(sch_rt)), tuple(sorted(pool_rt)),
           tuple(sorted(dma_rt)))
    if key in _NC_CACHE:
        return _NC_CACHE[key]

    W = N // N_CORES          # rows per core == column-group width
    RT = W // 128             # 128-row tiles per core
    KP = D // 256             # DoubleRow contraction chunks (256 rows each)
    NS = (W + 511) // 512     # 512-wide matmul subtiles per column group
    G = N_CORES               # column groups
    S = 4                     # strided DVE accumulators per group

    f8 = mybir.dt.float8e4
    bf16 = mybir.dt.bfloat16
    u16 = mybir.dt.uint16
    f32 = mybir.dt.float32
    DR = mybir.MatmulPerfMode.DoubleRow
    sch_rt = frozenset(sch_rt)
    pool_rt = frozenset(pool_rt)
    dma_rt = frozenset(dma_rt)
    off_rt = pool_rt | dma_rt

    nc = bacc.Bacc("TRN2", target_bir_lowering=False, debug=False,
                   num_devices=N_CORES)
    # [kp*128 + p, i*W + m] = img[m, kp*256 + i*128 + p]  (fp8, pre-packed)
    img8 = nc.dram_tensor("img8", [KP * 128, 2 * W], f8, kind="ExternalInput")
    # [kp*128 + p, gi*2W + i*W + m] = txt_rot[gi*W + m, kp*256 + i*128 + p]
    txt8 = nc.dram_tensor("txt8", [KP * 128, G * 2 * W], f8,
                          kind="ExternalInput")
    iden = nc.dram_tensor("iden", [128, 128], f32, kind="ExternalInput")
    out_col = nc.dram_tensor("out_col", [1, N], f32, kind="ExternalOutput")
    out_row = nc.dram_tensor("out_row", [128, RT], f32, kind="ExternalOutput")
    out_diag = nc.dram_tensor("out_diag", [128, RT], f32, kind="ExternalOutput")

    with tile.TileContext(nc) as tc:
        with (
            tc.tile_pool(name="imgp", bufs=1) as imgp,
            tc.tile_pool(name="txtp", bufs=2) as txtp,
            tc.tile_pool(name="cap", bufs=1) as cap,
            tc.tile_pool(name="csp", bufs=2) as csp,
            tc.tile_pool(name="smal", bufs=1) as smal,
            tc.tile_pool(name="scrp", bufs=4) as scrp,
        ):
            img_k = []
            for kp in range(KP):
                t = imgp.tile([128, 2, W], f8, name=f"img{kp}", tag=f"img{kp}")
                # img via the ACT hwdge queue so it overlaps the txt DMAs on
                # SP during the startup ramp
                nc.scalar.dma_start(t[:], img8[128 * kp:128 * (kp + 1), :])
                img_k.append(t)
            iden_t = smal.tile([128, 128], f32, name="iden_t")
            nc.scalar.dma_start(iden_t[:], iden[:])
            ones_t = smal.tile([128, 1], bf16, name="ones_t")
            nc.vector.memset(ones_t[:], 1.0)
            bias_t = smal.tile([128, 1], f32, name="bias_t")
            nc.vector.memset(bias_t[:], -C_OFF)

            def emit_rep(rep):
              diagS = smal.tile([128, RT], f32, name="diagS", tag="diagS")
              rowsumS = [smal.tile([128, G], f32, name=f"rows{rt}", tag=f"rows{rt}")
                         for rt in range(RT)]
              rowtot = smal.tile([128, RT], f32, name="rowtot", tag="rowtot")
              colfins = []

              with tc.tile_pool(name=f"psum{rep}", bufs=2, space="PSUM") as psp:
                  for gi in range(G):
                      txt_k = []
                      for kp in range(KP):
                          t = txtp.tile([128, 2, W], f8, name=f"txt{kp}",
                                        tag=f"txt{kp}")
                          nc.sync.dma_start(
                              t[:], txt8[128 * kp:128 * (kp + 1),
                                         2 * W * gi:2 * W * (gi + 1)])
                          txt_k.append(t)
                      # strided bf16 partial accumulators: cs[0] (== colfin)
                      # persists for the final ones-matmul; cs[1..3] recycle
                      cs = [cap.tile([128, W], bf16, name=f"colfin{gi}",
                                     tag=f"colfin{gi}")]
                      for si in range(1, S):
                          cs.append(csp.tile([128, W], bf16, name=f"cs{si}",
                                             tag=f"cs{si}"))
                      colfins.append(cs[0])
                      cs_used = [False] * S
                      cp = cp_used = None
                      if pool_rt:
                          cp = csp.tile([128, W], bf16, name="cp", tag="cp")
                          cp_used = [False]
                      cds = []
                      cd_used = []
                      if dma_rt:
                          cds = [csp.tile([128, W], bf16, name=f"cd{i}",
                                          tag=f"cd{i}") for i in range(2)]
                          cd_used = [False, False]
                      # software pipeline: reduce lags 1 tile, add lags 2
                      pend_red = []   # (rt, exv)
                      pend_add = []   # (rt, exv)

                      def drain(min_red, min_add):
                          while pend_red and len(pend_red) > min_red:
                              prt, pexv = pend_red.pop(0)
                              nc.vector.reduce_sum(
                                  rowsumS[prt][:, gi:gi + 1], pexv,
                                  axis=mybir.AxisListType.X)
                          while pend_add and len(pend_add) > min_add:
                              prt, pexv = pend_add.pop(0)
                              if prt in pool_rt:
                                  if not cp_used[0]:
                                      nc.gpsimd.tensor_copy(cp[:], pexv)
                                      cp_used[0] = True
                                  else:
                                      nc.gpsimd.tensor_add(cp[:], cp[:], pexv)
                              elif prt in dma_rt:
                                  di = (prt // 2) % 2
                                  if not cd_used[di]:
                                      # first link: plain DMA copy, no memset
                                      nc.gpsimd.dma_start(cds[di][:], pexv)
                                      cd_used[di] = True
                                  else:
                                      nc.gpsimd.dma_start(
                                          cds[di][:], pexv,
                                          accum_op=mybir.AluOpType.add)
                              else:
                                  si = prt % S
                                  if not cs_used[si]:
                                      nc.vector.tensor_copy(cs[si][:], pexv)
                                      cs_used[si] = True
                                  else:
                                      nc.vector.tensor_add(cs[si][:],
                                                           cs[si][:], pexv)
                      for rt in range(RT):
                          ps = psp.tile([128, W], f32, name="ps", tag="ps")
                          # kp outer / ns inner: 4 consecutive matmuls share
                          # the same stationary weights -> LDWEIGHTS amortized
                          for kp in range(KP):
                              for ns in range(NS):
                                  nw = min(512, W - 512 * ns)
                                  nc.tensor.matmul(
                                      ps[:, 512 * ns:512 * ns + nw],
                                      lhsT=img_k[kp][:, :, 128 * rt:128 * (rt + 1)],
                                      rhs=txt_k[kp][:, :, 512 * ns:512 * ns + nw],
                                      start=(kp == 0), stop=(kp == KP - 1),
                                      perf_mode=DR)
                          if gi == 0:
                              # diagonal logits live in this group's [rt] block
                              scr = scrp.tile([128, 128], f32, name="scr", tag="scr")
                              nc.vector.tensor_mul(
                                  scr[:], ps[:, 128 * rt:128 * (rt + 1)], iden_t[:])
                              nc.vector.reduce_sum(diagS[:, rt:rt + 1], scr[:],
                                                   axis=mybir.AxisListType.X)
                          # exp to SBUF (not in-place) so the PSUM bank frees
                          # as soon as the exp engine has read it
                          if rt in sch_rt:
                              exs = scrp.tile([128, W], u16, name="exs", tag="exs")
                              nc.vector.tensor_scalar(
                                  exs[:], ps[:], A16, B16 - A16 * C_OFF,
                                  mybir.AluOpType.mult, mybir.AluOpType.add)
                              exv = exs[:].bitcast(bf16)
                              pend_red.append((rt, exv))
                          else:
                              ex = scrp.tile([128, W], bf16, name="ex", tag="ex")
                              nc.scalar.activation(
                                  ex[:], ps[:], mybir.ActivationFunctionType.Exp,
                                  bias=bias_t[:], scale=1.0,
                                  accum_out=rowsumS[rt][:, gi:gi + 1])
                              exv = ex[:]
                          pend_add.append((rt, exv))
                          drain(min_red=1, min_add=2)
                      drain(min_red=0, min_add=0)
                      # combine the strided partials into cs[0] (pairwise so
                      # consecutive DVE ops stay independent)
                      if cs_used[1]:
                          nc.vector.tensor_add(cs[0][:], cs[0][:], cs[1][:])
                      if cs_used[3]:
                          nc.vector.tensor_add(cs[2][:], cs[2][:], cs[3][:])
                      if cd_used and cd_used[1]:
                          nc.vector.tensor_add(cds[0][:], cds[0][:], cds[1][:])
                      if cs_used[2]:
                          nc.vector.tensor_add(cs[0][:], cs[0][:], cs[2][:])
                      if pool_rt and cp_used[0]:
                          nc.vector.tensor_add(cs[0][:], cs[0][:], cp[:])
                      if cd_used and cd_used[0]:
                          nc.vector.tensor_add(cs[0][:], cs[0][:], cds[0][:])

              for rt in range(RT):
                  nc.vector.reduce_sum(rowtot[:, rt:rt + 1], rowsumS[rt][:],
                                       axis=mybir.AxisListType.X)
              nc.sync.dma_start(out_row[:], rowtot[:])
              nc.sync.dma_start(out_diag[:], diagS[:])

              with tc.tile_pool(name=f"cred{rep}", bufs=2, space="PSUM") as crp:
                  for gi in range(G):
                      cr = crp.tile([1, W], f32, name="cr", tag="cr")
                      for ns in range(NS):
                          nw = min(512, W - 512 * ns)
                          nc.tensor.matmul(
                              cr[:, 512 * ns:512 * ns + nw],
                              lhsT=ones_t[:],
                              rhs=colfins[gi][:, 512 * ns:512 * ns + nw],
                              start=True, stop=True)
                      # stage through SBUF on ACT: it is idle during this tail
                      crs = scrp.tile([1, W], f32, name="crs", tag="crs")
                      nc.scalar.copy(crs[:], cr[:])
                      nc.sync.dma_start(out_col[:, W * gi:W * (gi + 1)], crs[:])

            for rep in range(repeat):
                emit_rep(rep)

    nc.compile()
    _NC_CACHE[key] = nc
    return nc


def make_in_maps(image_features, text_features):
    img = np.asarray(image_features, dtype=np.float32)
    txt = np.asarray(text_features, dtype=np.float32)
    N, D = img.shape
    W = N // N_CORES
    KP = D // 256
    G = N_CORES
    f8np = ml_dtypes.float8_e4m3
    imgT8 = np.ascontiguousarray(img.T).astype(f8np)   # [D, N]
    txtT8 = np.ascontiguousarray(txt.T).astype(f8np)   # [D, N]
    iden = np.eye(128, dtype=np.float32)
    in_maps = []
    for k in range(N_CORES):
        # local image slab, DoubleRow-packed: [kp, p, i, m]
        islab = imgT8[:, W * k:W * (k + 1)]            # [D, W]
        ipk = islab.reshape(KP, 2, 128, W).transpose(0, 2, 1, 3)
        # rotate txt so local col j maps to global col (W*k + j) mod N,
        # then DoubleRow-pack per group: [kp, p, gi, i, m]
        tslab = np.roll(txtT8, -W * k, axis=1)         # [D, N]
        tpk = tslab.reshape(KP, 2, 128, G, W).transpose(0, 2, 3, 1, 4)
        in_maps.append({
            "img8": np.ascontiguousarray(ipk).reshape(KP * 128, 2 * W),
            "txt8": np.ascontiguousarray(tpk).reshape(KP * 128, 2 * G * W),
            "iden": iden,
        })
    return in_maps


def combine(results, N):
    W = N // N_CORES
    colsum = np.zeros(N, dtype=np.float64)
    s_row = 0.0
    s_diag = 0.0
    for k in range(N_CORES):
        r = results[k]
        colsum += np.roll(r["out_col"][0].astype(np.float64), W * k)
        s_row += np.log(r["out_row"].astype(np.float64)).sum()
        s_diag += r["out_diag"].astype(np.float64).sum()
    s_col = np.log(colsum).sum()
    loss = C_OFF + (0.5 * (s_row + s_col) - s_diag) / N
    return np.asarray(loss, dtype=np.float32)


def kernel(image_features, text_features):
    img = np.asarray(image_features)
    N, D = img.shape
    nc = build_nc(N, D)
    in_maps = make_in_maps(image_features, text_features)
    res = run_bass_kernel_spmd(nc, in_maps, core_ids=list(range(N_CORES)))
    return combine(res.results, N)
